# revision 1
# baseline (speedup 1.0000x reference)
"""Causal self-attention (B=2, T=2048, C=1024, H=16) on 8 TRN2 NeuronCores.

Sharding: tensor-parallel over heads (2 heads/core) for QKV projection and
attention; AllToAll converts the head-sharded attention output into a
sequence-sharded layout; each core then computes its 512-row slice of the
output projection. Host only slices/casts inputs and concatenates outputs.

Device math in bf16 with fp32 PSUM accumulation:
  - x is pre-transposed on host to xT [C, B*T] (bf16) so every matmul
    contraction has channels on the partition axis.
  - Scores are built transposed (S^T [keys, queries]) so softmax
    normalization sums arrive for free from a ones-augmented P^T @ [V|1]
    matmul, and no on-device transposes are needed anywhere.
  - exp on ScalarE (fp32-accurate LUT); no max-subtraction needed since
    scores are O(+-8).
  - AllToAll granularity is a 1024-query superchunk (two 512-query attention
    chunks), halving collective count; the output projection packs the two
    64-row strips into a single 128-col lhsT so the PE array runs full width.
"""
import os
import math
import threading

import numpy as np
import ml_dtypes

import concourse.bass as bass
import concourse.tile as tile
from concourse import mybir, bacc, bass_utils

B, T, C, H = 2, 2048, 1024, 16
D = C // H                 # 64
NCORES = 8
HPC = H // NCORES          # heads per core = 2
HC = HPC * D               # head-channels per core = 128
BT = B * T                 # 4096
TQ = 512                   # query chunk
TKT = 128                  # key tile
ROWS = BT // NCORES        # output rows per core = 512
SM_SCALE = 1.0 / math.sqrt(D)

F32 = mybir.dt.float32
BF16 = mybir.dt.bfloat16
BF16_NP = ml_dtypes.bfloat16


def _build_program():
    nc = bacc.Bacc("TRN2", target_bir_lowering=False, debug=False,
                   num_devices=NCORES)
    xt = nc.dram_tensor("xt", [C, BT], BF16, kind="ExternalInput").ap()
    wqkv = nc.dram_tensor("wqkv", [C, 3 * HC], BF16, kind="ExternalInput").ap()
    wproj = nc.dram_tensor("wproj", [C, C], BF16, kind="ExternalInput").ap()
    bq = nc.dram_tensor("bq", [HC, 1], F32, kind="ExternalInput").ap()
    bk = nc.dram_tensor("bk", [HC, 1], F32, kind="ExternalInput").ap()
    bv = nc.dram_tensor("bv", [1, HC], BF16, kind="ExternalInput").ap()
    bproj = nc.dram_tensor("bproj", [1, C], BF16, kind="ExternalInput").ap()
    masks = nc.dram_tensor("masks", [TQ // TKT, TKT, TQ], BF16,
                           kind="ExternalInput").ap()
    outp = nc.dram_tensor("out", [ROWS, C], BF16, kind="ExternalOutput").ap()

    KT = C // 128          # 8 contraction tiles over channels
    NCH = BT // TQ         # 8 T-chunks over B*T
    SPC = TQ // D          # 8 strips of 64 rows per chunk (one per core)
    NSC = NCH // 2         # 4 superchunks (a2a granularity, 1024 queries)

    with tile.TileContext(nc) as tc:
        with (
            tc.tile_pool(name="consts", bufs=1) as consts,
            tc.tile_pool(name="xpool", bufs=2) as xpool,
            tc.tile_pool(name="ppool", bufs=8) as ppool,
            tc.tile_pool(name="npool", bufs=2) as npool,
            tc.tile_pool(name="opool", bufs=2) as opool,
            tc.tile_pool(name="ps_o", bufs=2, space="PSUM") as ps_o,
            tc.tile_pool(name="dram", bufs=1, space="DRAM") as dram,
        ):
            # ---- stage 0: weights & constants ----
            # wqkv on the scalar-engine queue so the sync queue starts
            # streaming stage-1 x tiles immediately
            wqkv_sb = []
            for kt in range(KT):
                w1 = consts.tile([128, 3 * HC], BF16, name=f"wqkv_sb{kt}")
                nc.scalar.dma_start(out=w1, in_=wqkv[128 * kt:128 * (kt + 1), :])
                wqkv_sb.append(w1)
            # big weights not needed until ~100us in: keep them off the SP
            # HWDGE queue so the stage-1 stream starts immediately
            wproj_sb = []
            for kt in range(KT):
                w2 = consts.tile([128, C], BF16, name=f"wproj_sb{kt}")
                nc.gpsimd.dma_start(out=w2, in_=wproj[128 * kt:128 * (kt + 1), :])
                wproj_sb.append(w2)
            ones_sb = consts.tile([1, 128], BF16, name="ones_sb")
            nc.vector.memset(ones_sb, 1.0)
            bq_sb = consts.tile([HC, 1], F32, name="bq_sb")
            nc.sync.dma_start(out=bq_sb, in_=bq)
            bk_sb = consts.tile([HC, 1], F32, name="bk_sb")
            nc.sync.dma_start(out=bk_sb, in_=bk)
            bv_sb = consts.tile([1, HC], BF16, name="bv_sb")
            nc.sync.dma_start(out=bv_sb, in_=bv)
            bproj_sb = consts.tile([1, C], BF16, name="bproj_sb")
            nc.sync.dma_start(out=bproj_sb, in_=bproj)
            masks_sb = consts.tile([TKT, TQ // TKT, TQ], BF16, name="masks_sb")
            nc.gpsimd.dma_start(out=masks_sb, in_=masks.rearrange("r p q -> p r q"))

            qT_b = [consts.tile([HC, T], BF16, name=f"qT_sb{b}")
                    for b in range(B)]
            kT_b = [consts.tile([HC, T], BF16, name=f"kT_sb{b}")
                    for b in range(B)]
            v_sb = [consts.tile([128, 2 * (D + 1)], BF16, name=f"v_sb{tt}")
                    for tt in range(BT // 128)]

            # per-superchunk exchange buffers: block s = queries
            # [64s, 64s+64) of each of the two chunks, owned by core s
            a2a_in = [dram.tile([NCORES, HC, 2, D], BF16, name=f"a2a_in{c}")
                      for c in range(NSC)]
            a2a_out = [dram.tile([NCORES, HC, 2, D], BF16, name=f"a2a_out{c}")
                       for c in range(NSC)]
            # the very last superchunk exchanges per-chunk so the first half
            # of its output projection hides under the final AllToAll
            a2a_in_f = [dram.tile([NCORES, HC, D], BF16, name=f"a2a_inf{j}")
                        for j in range(2)]
            a2a_out_f = [dram.tile([NCORES, HC, D], BF16, name=f"a2a_outf{j}")
                         for j in range(2)]

            def stage4(sc):
                """Output projection for one 128-row superchunk strip."""
                yy = opool.tile([128, KT, 128], BF16, tag="ylhs", name="ylhs")
                nc.sync.dma_start(
                    out=yy, in_=a2a_out[sc].rearrange("k p j q -> p k (j q)"))
                for n in range(C // TQ):
                    po = ps_o.tile([128, TQ], F32, tag="po", name="po")
                    for kt in range(KT):
                        nc.tensor.matmul(
                            po,
                            lhsT=yy[:, kt, :],
                            rhs=wproj_sb[kt][:, TQ * n:TQ * (n + 1)],
                            start=(kt == 0), stop=False)
                    nc.tensor.matmul(
                        po, lhsT=ones_sb,
                        rhs=bproj_sb[:, TQ * n:TQ * (n + 1)],
                        start=False, stop=True)
                    osb = opool.tile([128, TQ], BF16, tag="osb", name="osb")
                    nc.vector.tensor_copy(out=osb, in_=po)
                    nc.gpsimd.dma_start(
                        out=outp[128 * sc:128 * (sc + 1),
                                 TQ * n:TQ * (n + 1)],
                        in_=osb)

            def stage4_final_half(yyf, pos, half):
                c0, c1 = D * half, D * (half + 1)
                nc.sync.dma_start(
                    out=yyf[:, :, c0:c1],
                    in_=a2a_out_f[half].rearrange("k p q -> p k q"))
                for n in range(C // TQ):
                    if half == 0:
                        pos.append(ps_o.tile([128, TQ], F32, tag="po",
                                             name="po"))
                    po = pos[n]
                    for kt in range(KT):
                        nc.tensor.matmul(
                            po[c0:c1, :], lhsT=yyf[:, kt, c0:c1],
                            rhs=wproj_sb[kt][:, TQ * n:TQ * (n + 1)],
                            start=(kt == 0), stop=False)
                    nc.tensor.matmul(
                        po[c0:c1, :], lhsT=ones_sb[:, c0:c1],
                        rhs=bproj_sb[:, TQ * n:TQ * (n + 1)],
                        start=False, stop=True)
                    if half == 1:
                        sc = 2 * (B - 1)
                        osb = opool.tile([128, TQ], BF16, tag="osb",
                                         name="osb")
                        nc.vector.tensor_copy(out=osb, in_=po)
                        nc.gpsimd.dma_start(
                            out=outp[128 * sc:128 * (sc + 1),
                                     TQ * n:TQ * (n + 1)],
                            in_=osb)

            # stage-4 emission is deferred until the producing AllToAll
            # has certainly completed, so its instructions never sit in the
            # engine FIFOs blocking independent attention/QKV work behind a
            # collective wait
            pending = []
            for b in range(B):
                # ---- stage 1: QKV projection for this batch ----
                with (
                    tc.tile_pool(name=f"ps_qk{b}", bufs=3, space="PSUM") as ps_qk,
                    tc.tile_pool(name=f"ps_v{b}", bufs=2, space="PSUM") as ps_v,
                ):
                    for cl in range(NCH // B):
                        if cl == 2 and pending:
                            stage4(pending.pop(0))
                        c = (NCH // B) * b + cl
                        xt_t = []
                        ps_q = ps_qk.tile([HC, TQ], F32, tag="qk")
                        ps_k = ps_qk.tile([HC, TQ], F32, tag="qk")
                        for kt in range(KT):
                            xx = xpool.tile([128, TQ], BF16, tag=f"xt{kt}")
                            nc.sync.dma_start(
                                out=xx,
                                in_=xt[128 * kt:128 * (kt + 1),
                                       TQ * c:TQ * (c + 1)])
                            xt_t.append(xx)
                            nc.tensor.matmul(
                                ps_q, lhsT=wqkv_sb[kt][:, 0:HC], rhs=xx,
                                start=(kt == 0), stop=(kt == KT - 1))
                            nc.tensor.matmul(
                                ps_k, lhsT=wqkv_sb[kt][:, HC:2 * HC], rhs=xx,
                                start=(kt == 0), stop=(kt == KT - 1))
                        for ps, dst, bias in (
                            (ps_q, qT_b[b], bq_sb),
                            (ps_k, kT_b[b], bk_sb),
                        ):
                            nc.vector.tensor_scalar_add(
                                out=dst[:, TQ * cl:TQ * (cl + 1)], in0=ps,
                                scalar1=bias)
                        # V (natural layout, ones-augmented)
                        for s in range(TQ // 128):
                            tt = 4 * c + s
                            ps = ps_v.tile([128, HC], F32, tag="v")
                            for kt in range(KT):
                                nc.tensor.matmul(
                                    ps,
                                    lhsT=xt_t[kt][:, 128 * s:128 * (s + 1)],
                                    rhs=wqkv_sb[kt][:, 2 * HC:3 * HC],
                                    start=(kt == 0), stop=False)
                            nc.tensor.matmul(ps, lhsT=ones_sb, rhs=bv_sb,
                                             start=False, stop=True)
                            vt = v_sb[tt]
                            nc.vector.tensor_copy(
                                out=vt.rearrange("p (g e) -> p g e",
                                                 g=2)[:, :, 0:D],
                                in_=ps.rearrange("p (g d) -> p g d", g=2))
                            nc.vector.memset(vt[:, D:D + 1], 1.0)
                            nc.vector.memset(vt[:, 2 * D + 1:2 * D + 2], 1.0)

                # ---- stage 2: attention for this batch, smallest chunk
                # first; a superchunk's exchange + output projection fire
                # after every second chunk and hide under later chunks'
                # (larger) attention ----
                with (
                    tc.tile_pool(name=f"ps_s{b}", bufs=4, space="PSUM") as ps_s,
                    tc.tile_pool(name=f"ps_y{b}", bufs=1, space="PSUM") as ps_y,
                ):
                    ytc = None
                    # big superchunk first: its exchange hides under the
                    # small chunks' attention; the small superchunk's
                    # exchange is the only one exposed at the batch tail
                    yyf = None
                    pos = []
                    for jl in (2, 3, 0, 1):
                        last_sc = (b == B - 1 and jl < 2)
                        if b == B - 1 and jl == 3 and pending:
                            stage4(pending.pop(0))
                        if b == B - 1 and jl == 1:
                            if pending:
                                stage4(pending.pop(0))
                            nc.gpsimd.collective_compute(
                                "AllToAll", mybir.AluOpType.bypass,
                                replica_groups=[list(range(NCORES))],
                                ins=[a2a_in_f[0].opt()],
                                outs=[a2a_out_f[0].opt()])
                        sc = 2 * b + jl // 2
                        j = jl % 2
                        if j == 0:
                            ytc = [npool.tile([D, 2, TQ], BF16, tag=f"ytc{h}",
                                              name=f"ytc{h}")
                                   for h in range(HPC)]
                        q0 = TQ * jl
                        nkt = (TQ // TKT) * (jl + 1)
                        y_ps = [ps_y.tile([D + 1, TQ], F32, tag=f"y{h}",
                                          name=f"y_ps{h}")
                                for h in range(HPC)]
                        # diagonal key tiles first (r=0 is full-width and
                        # opens the PSUM accumulation), then the full
                        # history tiles; diagonal tiles only touch columns
                        # >= TKT*r (the rest is causally masked away)
                        dg = TQ // TKT
                        if jl == 0:
                            kts = list(range(nkt))
                        else:
                            kts = (list(range(dg * jl, nkt))
                                   + list(range(dg * jl)))
                        pts = {}
                        for kt in kts:
                            k0 = TKT * kt
                            r = kt - dg * jl
                            c0 = TKT * r if r >= 1 else 0
                            pt_pair = []
                            for h in range(HPC):
                                hp = D * h
                                ss = ps_s.tile([TKT, TQ], F32, tag="s",
                                               name=f"ss{h}")
                                nc.tensor.matmul(
                                    ss[:, c0:],
                                    lhsT=kT_b[b][hp:hp + D, k0:k0 + TKT],
                                    rhs=qT_b[b][hp:hp + D, q0 + c0:q0 + TQ],
                                    start=True, stop=True)
                                pt = ppool.tile([TKT, TQ], BF16, tag=f"pt{h}",
                                                name=f"pt{h}")
                                nc.scalar.activation(
                                    out=pt[:, c0:], in_=ss[:, c0:],
                                    func=mybir.ActivationFunctionType.Exp)
                                if jl == 0 and c0 > 0:
                                    # AV stays full-width for jl==0
                                    nc.vector.memset(pt[:, 0:c0], 0.0)
                                if r >= 0:
                                    m0 = TKT * r
                                    nc.vector.tensor_mul(
                                        pt[:, m0:m0 + TKT],
                                        pt[:, m0:m0 + TKT],
                                        masks_sb[:, r, m0:m0 + TKT])
                                pt_pair.append(pt)
                            pts[kt] = pt_pair
                        for i, kt in enumerate(kts):
                            vt = v_sb[(T // 128) * b + kt]
                            r = kt - dg * jl
                            c0 = TKT * r if (jl > 0 and r >= 1) else 0
                            del r
                            for h in range(HPC):
                                nc.tensor.matmul(
                                    y_ps[h][:, c0:],
                                    lhsT=vt[:, (D + 1) * h:(D + 1) * (h + 1)],
                                    rhs=pts[kt][h][:, c0:],
                                    start=(i == 0), stop=(i == len(kts) - 1))
                        for h in range(HPC):
                            den = npool.tile([1, TQ], F32, tag="den")
                            nc.vector.tensor_copy(out=den,
                                                  in_=y_ps[h][D:D + 1, :])
                            recip = npool.tile([1, TQ], F32, tag="recip")
                            nc.vector.reciprocal_approx_fast(recip, den)
                            recip_b = npool.tile([D, TQ], F32, tag="recipb")
                            nc.gpsimd.partition_broadcast(recip_b, recip)
                            nc.vector.tensor_mul(ytc[h][:, j, :],
                                                 y_ps[h][0:D, :], recip_b)
                            if last_sc:
                                nc.sync.dma_start(
                                    out=a2a_in_f[j][:, D * h:D * (h + 1), :]
                                        .rearrange("s p q -> p s q"),
                                    in_=ytc[h][:, j, :]
                                        .rearrange("p (s q) -> p s q", s=SPC))
                            else:
                                nc.sync.dma_start(
                                    out=a2a_in[sc][:, D * h:D * (h + 1), j, :]
                                        .rearrange("s p q -> p s q"),
                                    in_=ytc[h][:, j, :]
                                        .rearrange("p (s q) -> p s q", s=SPC))
                        if j == 1 and not last_sc:
                            nc.gpsimd.collective_compute(
                                "AllToAll", mybir.AluOpType.bypass,
                                replica_groups=[list(range(NCORES))],
                                ins=[a2a_in[sc].opt()],
                                outs=[a2a_out[sc].opt()])
                            pending.append(sc)
                        elif j == 1:
                            yyf = opool.tile([128, KT, 128], BF16,
                                             tag="ylhs", name="ylhsf")
                            stage4_final_half(yyf, pos, 0)
                            nc.gpsimd.collective_compute(
                                "AllToAll", mybir.AluOpType.bypass,
                                replica_groups=[list(range(NCORES))],
                                ins=[a2a_in_f[1].opt()],
                                outs=[a2a_out_f[1].opt()])
                            stage4_final_half(yyf, pos, 1)

            for sc in pending:
                stage4(sc)

    nc.compile()
    return nc


_lock = threading.Lock()
_cached_nc = None
last_results = None  # BassKernelResults of the most recent kernel() call


def _get_program():
    global _cached_nc
    with _lock:
        if _cached_nc is None:
            _cached_nc = _build_program()
    return _cached_nc


def _host_inputs(x, W_qkv, b_qkv, W_proj, b_proj):
    bf = lambda a: np.ascontiguousarray(a).astype(BF16_NP)
    x = np.asarray(x, dtype=np.float32)
    W_qkv = np.asarray(W_qkv, dtype=np.float32)
    b_qkv = np.asarray(b_qkv, dtype=np.float32)
    W_proj = np.asarray(W_proj, dtype=np.float32)
    b_proj = np.asarray(b_proj, dtype=np.float32)

    xt = bf(x.reshape(BT, C).T)                     # [C, BT]
    wproj = bf(W_proj)                              # [C, C]
    bproj = bf(b_proj.reshape(1, C))
    r = np.arange(TQ // TKT)[:, None, None]
    k = np.arange(TKT)[None, :, None]
    q = np.arange(TQ)[None, None, :]
    masks = ((k + TKT * r) <= q).astype(BF16_NP)    # [4, 128, 512]

    in_maps = []
    for i in range(NCORES):
        sel = slice(HC * i, HC * (i + 1))
        wq = W_qkv[:, sel]
        wk = W_qkv[:, C + HC * i:C + HC * (i + 1)] * SM_SCALE
        wv = W_qkv[:, 2 * C + HC * i:2 * C + HC * (i + 1)]
        in_maps.append({
            "xt": xt,
            "wqkv": bf(np.concatenate([wq, wk, wv], axis=1)),
            "wproj": wproj,
            "bq": np.ascontiguousarray(
                b_qkv[sel].reshape(HC, 1)).astype(np.float32),
            "bk": np.ascontiguousarray(
                (b_qkv[C + HC * i:C + HC * (i + 1)] * SM_SCALE)
                .reshape(HC, 1)).astype(np.float32),
            "bv": b_qkv[2 * C + HC * i:2 * C + HC * (i + 1)]
                .reshape(1, HC).astype(BF16_NP),
            "bproj": bproj,
            "masks": masks,
        })
    return in_maps


def kernel(x, W_qkv, b_qkv, W_proj, b_proj):
    global last_results
    nc = _get_program()
    in_maps = _host_inputs(x, W_qkv, b_qkv, W_proj, b_proj)
    trace = bool(int(os.environ.get("KERNEL_TRACE", "0")))
    res = bass_utils.run_bass_kernel_spmd(
        nc, in_maps, core_ids=list(range(NCORES)), trace=trace)
    last_results = res
    # core s's output rows are strip s (64 rows) of every 512-row chunk
    arr = np.stack([res.results[s]["out"].reshape(BT // TQ, D, C)
                    for s in range(NCORES)], axis=1)   # [chunk, core, 64, C]
    return np.ascontiguousarray(arr.reshape(B, T, C)).astype(np.float32)



# revision 2
# speedup vs baseline: 1.2093x; 1.2093x over previous
"""Causal self-attention (B=2, T=2048, C=1024, H=16) on 8 TRN2 NeuronCores.

Sharding: tensor-parallel over heads (2 heads/core) for QKV projection and
attention; AllToAll converts the head-sharded attention output into a
sequence-sharded layout; each core then computes its 512-row slice of the
output projection. Host only slices/casts inputs and concatenates outputs.

Device math in bf16 with fp32 PSUM accumulation:
  - x is pre-transposed on host to xT [C, B*T] (bf16) so every matmul
    contraction has channels on the partition axis.
  - Scores are built transposed (S^T [keys, queries]) so softmax
    normalization sums arrive for free from a ones-augmented P^T @ [V|1]
    matmul, and no on-device transposes are needed anywhere.
  - The two heads' score matmuls write adjacent PSUM banks of one
    [128, 2, TQ] tile (row-tiled to array rows 0-63 / 64-127 so they can
    overlap in the PE), and ONE exp activation covers both heads per key
    tile, halving the ScalarE per-call overhead.
  - exp on ScalarE (fp32-accurate LUT); no max-subtraction needed since
    scores are O(+-8).
  - AllToAll granularity is a 1024-query superchunk (two 512-query attention
    chunks), halving collective count; the output projection packs the two
    64-row strips into a single 128-col lhsT so the PE array runs full width.
  - Deferred stage-4 (collective-consuming) work carries tile_wait_until
    floors so the Tile scheduler cannot hoist it ahead of independent
    QKV/attention work in the engine queues (head-of-line blocking on the
    collective-done semaphore starved the PE for ~28us otherwise).
  - x / wqkv / wproj land via one DMA each per chunk/tensor (the HWDGE
    fans a single descriptor stream across all 16 HW queues), cutting
    sequencer DIRECT2D issue time ~8x.
"""
import os
import math
import threading

import numpy as np
import ml_dtypes

import concourse.bass as bass
import concourse.tile as tile
from concourse import mybir, bacc, bass_utils

B, T, C, H = 2, 2048, 1024, 16
D = C // H                 # 64
NCORES = 8
HPC = H // NCORES          # heads per core = 2
HC = HPC * D               # head-channels per core = 128
BT = B * T                 # 4096
TQ = 512                   # query chunk
TKT = 128                  # key tile
ROWS = BT // NCORES        # output rows per core = 512
SM_SCALE = 1.0 / math.sqrt(D)

F32 = mybir.dt.float32
BF16 = mybir.dt.bfloat16
BF16_NP = ml_dtypes.bfloat16

# Tile-scheduler modeled-time floors (ms) for deferred stage-4 bodies.
S4_FLOOR = {1: 0.130, 0: 0.170, 3: 0.230, 2: 0.260}
S4F_FLOOR = {0: 0.240, 1: 0.250}


def _build_program():
    nc = bacc.Bacc("TRN2", target_bir_lowering=False, debug=False,
                   num_devices=NCORES)
    xt = nc.dram_tensor("xt", [C, BT], BF16, kind="ExternalInput").ap()
    wqkv = nc.dram_tensor("wqkv", [C, 3 * HC], BF16, kind="ExternalInput").ap()
    wproj = nc.dram_tensor("wproj", [C, C], BF16, kind="ExternalInput").ap()
    bq = nc.dram_tensor("bq", [HC, 1], F32, kind="ExternalInput").ap()
    bk = nc.dram_tensor("bk", [HC, 1], F32, kind="ExternalInput").ap()
    bv = nc.dram_tensor("bv", [1, HC], BF16, kind="ExternalInput").ap()
    bproj = nc.dram_tensor("bproj", [1, C], BF16, kind="ExternalInput").ap()
    masks = nc.dram_tensor("masks", [TQ // TKT, TKT, TQ], BF16,
                           kind="ExternalInput").ap()
    outp = nc.dram_tensor("out", [ROWS, C], BF16, kind="ExternalOutput").ap()

    KT = C // 128          # 8 contraction tiles over channels
    NCH = BT // TQ         # 8 T-chunks over B*T
    SPC = TQ // D          # 8 strips of 64 rows per chunk (one per core)
    NSC = NCH // 2         # 4 superchunks (a2a granularity, 1024 queries)

    with tile.TileContext(nc) as tc:
        with (
            tc.tile_pool(name="consts", bufs=1) as consts,
            tc.tile_pool(name="xpool", bufs=2) as xpool,
            tc.tile_pool(name="ppool", bufs=8) as ppool,
            tc.tile_pool(name="npool", bufs=2) as npool,
            tc.tile_pool(name="opool", bufs=2) as opool,
            tc.tile_pool(name="ps_o", bufs=2, space="PSUM") as ps_o,
            tc.tile_pool(name="dram", bufs=1, space="DRAM") as dram,
        ):
            # ---- stage 0: weights & constants ----
            # wqkv on the scalar-engine queue so the sync queue starts
            # streaming stage-1 x tiles immediately; one DMA per tensor
            wqkv_sb = consts.tile([128, KT, 3 * HC], BF16, name="wqkv_sb")
            nc.scalar.dma_start(out=wqkv_sb,
                                in_=wqkv.rearrange("(k p) n -> p k n", p=128))
            # big weights not needed until ~100us in: keep them off the SP
            # HWDGE queue so the stage-1 stream starts immediately
            wproj_sb = consts.tile([128, KT, C], BF16, name="wproj_sb")
            nc.gpsimd.dma_start(out=wproj_sb,
                                in_=wproj.rearrange("(k p) n -> p k n", p=128))
            ones_sb = consts.tile([1, 128], BF16, name="ones_sb")
            nc.vector.memset(ones_sb, 1.0)
            bq_sb = consts.tile([HC, 1], F32, name="bq_sb")
            nc.scalar.dma_start(out=bq_sb, in_=bq)
            bk_sb = consts.tile([HC, 1], F32, name="bk_sb")
            nc.scalar.dma_start(out=bk_sb, in_=bk)
            bv_sb = consts.tile([1, HC], BF16, name="bv_sb")
            nc.scalar.dma_start(out=bv_sb, in_=bv)
            bproj_sb = consts.tile([1, C], BF16, name="bproj_sb")
            nc.scalar.dma_start(out=bproj_sb, in_=bproj)
            masks_sb = consts.tile([TKT, TQ // TKT, TQ], BF16, name="masks_sb")
            nc.gpsimd.dma_start(out=masks_sb, in_=masks.rearrange("r p q -> p r q"))

            qT_b = [consts.tile([HC, T], BF16, name=f"qT_sb{b}")
                    for b in range(B)]
            kT_b = [consts.tile([HC, T], BF16, name=f"kT_sb{b}")
                    for b in range(B)]
            v_sb = [consts.tile([128, 2 * (D + 1)], BF16, name=f"v_sb{tt}")
                    for tt in range(BT // 128)]

            # per-superchunk exchange buffers: block s = queries
            # [64s, 64s+64) of each of the two chunks, owned by core s
            a2a_in = [dram.tile([NCORES, HC, 2, D], BF16, name=f"a2a_in{c}")
                      for c in range(NSC)]
            a2a_out = [dram.tile([NCORES, HC, 2, D], BF16, name=f"a2a_out{c}")
                       for c in range(NSC)]
            # the very last superchunk exchanges per-chunk so the first half
            # of its output projection hides under the final AllToAll
            a2a_in_f = [dram.tile([NCORES, HC, D], BF16, name=f"a2a_inf{j}")
                        for j in range(2)]
            a2a_out_f = [dram.tile([NCORES, HC, D], BF16, name=f"a2a_outf{j}")
                         for j in range(2)]

            def stage4(sc):
                """Output projection for one 128-row superchunk strip."""
                yy = opool.tile([128, KT, 128], BF16, tag="ylhs", name="ylhs")
                nc.sync.dma_start(
                    out=yy, in_=a2a_out[sc].rearrange("k p j q -> p k (j q)"))
                for n in range(C // TQ):
                    po = ps_o.tile([128, TQ], F32, tag="po", name="po")
                    for kt in range(KT):
                        nc.tensor.matmul(
                            po,
                            lhsT=yy[:, kt, :],
                            rhs=wproj_sb[:, kt, TQ * n:TQ * (n + 1)],
                            start=(kt == 0), stop=False)
                    nc.tensor.matmul(
                        po, lhsT=ones_sb,
                        rhs=bproj_sb[:, TQ * n:TQ * (n + 1)],
                        start=False, stop=True)
                    osb = opool.tile([128, TQ], BF16, tag="osb", name="osb")
                    nc.vector.tensor_copy(out=osb, in_=po)
                    nc.gpsimd.dma_start(
                        out=outp[128 * sc:128 * (sc + 1),
                                 TQ * n:TQ * (n + 1)],
                        in_=osb)

            def stage4_final_half(yyf, pos, half):
                c0, c1 = D * half, D * (half + 1)
                nc.sync.dma_start(
                    out=yyf[:, :, c0:c1],
                    in_=a2a_out_f[half].rearrange("k p q -> p k q"))
                for n in range(C // TQ):
                    if half == 0:
                        pos.append(ps_o.tile([128, TQ], F32, tag="po",
                                             name="po"))
                    po = pos[n]
                    for kt in range(KT):
                        nc.tensor.matmul(
                            po[c0:c1, :], lhsT=yyf[:, kt, c0:c1],
                            rhs=wproj_sb[:, kt, TQ * n:TQ * (n + 1)],
                            start=(kt == 0), stop=False)
                    nc.tensor.matmul(
                        po[c0:c1, :], lhsT=ones_sb[:, c0:c1],
                        rhs=bproj_sb[:, TQ * n:TQ * (n + 1)],
                        start=False, stop=True)
                    if half == 1:
                        sc = 2 * (B - 1)
                        osb = opool.tile([128, TQ], BF16, tag="osb",
                                         name="osb")
                        nc.vector.tensor_copy(out=osb, in_=po)
                        nc.gpsimd.dma_start(
                            out=outp[128 * sc:128 * (sc + 1),
                                     TQ * n:TQ * (n + 1)],
                            in_=osb)

            # stage-4 bodies are emitted late AND carry modeled-time floors
            # (tile_wait_until) so the scheduler orders them behind the
            # independent QKV/attention work in every engine queue; their
            # collective-done waits then never sit at a queue head blocking
            # unrelated traffic
            pending = []

            def pop_stage4():
                sc = pending.pop(0)
                with tc.tile_wait_until(S4_FLOOR[sc]):
                    stage4(sc)

            for b in range(B):
                # ---- stage 1: QKV projection for this batch ----
                with (
                    tc.tile_pool(name=f"ps_qk{b}", bufs=3, space="PSUM") as ps_qk,
                    tc.tile_pool(name=f"ps_v{b}", bufs=2, space="PSUM") as ps_v,
                ):
                    for cl in range(NCH // B):
                        if cl == 2 and pending:
                            pop_stage4()
                        c = (NCH // B) * b + cl
                        xx = xpool.tile([128, KT, TQ], BF16, tag="xt")
                        nc.sync.dma_start(
                            out=xx,
                            in_=xt.rearrange("(k p) q -> p k q", p=128)
                                [:, :, TQ * c:TQ * (c + 1)])
                        ps_q = ps_qk.tile([HC, TQ], F32, tag="qk")
                        ps_k = ps_qk.tile([HC, TQ], F32, tag="qk")
                        for kt in range(KT):
                            nc.tensor.matmul(
                                ps_q, lhsT=wqkv_sb[:, kt, 0:HC],
                                rhs=xx[:, kt, :],
                                start=(kt == 0), stop=(kt == KT - 1))
                            nc.tensor.matmul(
                                ps_k, lhsT=wqkv_sb[:, kt, HC:2 * HC],
                                rhs=xx[:, kt, :],
                                start=(kt == 0), stop=(kt == KT - 1))
                        for ps, dst, bias in (
                            (ps_q, qT_b[b], bq_sb),
                            (ps_k, kT_b[b], bk_sb),
                        ):
                            nc.vector.tensor_scalar_add(
                                out=dst[:, TQ * cl:TQ * (cl + 1)], in0=ps,
                                scalar1=bias)
                        # V (natural layout, ones-augmented)
                        for s in range(TQ // 128):
                            tt = 4 * c + s
                            ps = ps_v.tile([128, HC], F32, tag="v")
                            for kt in range(KT):
                                nc.tensor.matmul(
                                    ps,
                                    lhsT=xx[:, kt, 128 * s:128 * (s + 1)],
                                    rhs=wqkv_sb[:, kt, 2 * HC:3 * HC],
                                    start=(kt == 0), stop=False)
                            nc.tensor.matmul(ps, lhsT=ones_sb, rhs=bv_sb,
                                             start=False, stop=True)
                            vt = v_sb[tt]
                            nc.vector.tensor_copy(
                                out=vt.rearrange("p (g e) -> p g e",
                                                 g=2)[:, :, 0:D],
                                in_=ps.rearrange("p (g d) -> p g d", g=2))
                            nc.vector.memset(vt[:, D:D + 1], 1.0)
                            nc.vector.memset(vt[:, 2 * D + 1:2 * D + 2], 1.0)

                # ---- stage 2: attention for this batch; a superchunk's
                # exchange + output projection fire after every second chunk
                # and hide under later chunks' (larger) attention ----
                with (
                    tc.tile_pool(name=f"ps_s{b}", bufs=2, space="PSUM") as ps_s,
                    tc.tile_pool(name=f"ps_y{b}", bufs=1, space="PSUM") as ps_y,
                ):
                    ytc = None
                    # big superchunk first: its exchange hides under the
                    # small chunks' attention; the small superchunk's
                    # exchange is the only one exposed at the batch tail
                    yyf = None
                    pos = []
                    for jl in (2, 3, 0, 1):
                        last_sc = (b == B - 1 and jl < 2)
                        if b == B - 1 and jl == 3 and pending:
                            pop_stage4()
                        if b == B - 1 and jl == 1:
                            if pending:
                                pop_stage4()
                            nc.gpsimd.collective_compute(
                                "AllToAll", mybir.AluOpType.bypass,
                                replica_groups=[list(range(NCORES))],
                                ins=[a2a_in_f[0].opt()],
                                outs=[a2a_out_f[0].opt()])
                        sc = 2 * b + jl // 2
                        j = jl % 2
                        if j == 0:
                            ytc = [npool.tile([D, 2, TQ], BF16, tag=f"ytc{h}",
                                              name=f"ytc{h}")
                                   for h in range(HPC)]
                        q0 = TQ * jl
                        nkt = (TQ // TKT) * (jl + 1)
                        y_ps = [ps_y.tile([D + 1, TQ], F32, tag=f"y{h}",
                                          name=f"y_ps{h}")
                                for h in range(HPC)]
                        # diagonal key tiles first (r=0 is full-width and
                        # opens the PSUM accumulation), then the full
                        # history tiles; diagonal tiles only touch columns
                        # >= TKT*r (the rest is causally masked away)
                        dg = TQ // TKT
                        if jl == 0:
                            kts = list(range(nkt))
                        else:
                            kts = (list(range(dg * jl, nkt))
                                   + list(range(dg * jl)))
                        pts = {}
                        for kt in kts:
                            k0 = TKT * kt
                            r = kt - dg * jl
                            c0 = TKT * r if r >= 1 else 0
                            # both heads' scores into adjacent PSUM banks of
                            # one tile; array rows 0-63 (h0) / 64-127 (h1)
                            # via auto tile_position so the pair can overlap
                            ss = ps_s.tile([TKT, 2, TQ], F32, tag="s",
                                           name="ss")
                            for h in range(HPC):
                                hp = D * h
                                nc.tensor.matmul(
                                    ss[:, h, c0:],
                                    lhsT=kT_b[b][hp:hp + D, k0:k0 + TKT],
                                    rhs=qT_b[b][hp:hp + D, q0 + c0:q0 + TQ],
                                    start=True, stop=True)
                            pt = ppool.tile([TKT, 2, TQ], BF16, tag="pt",
                                            name="pt")
                            # ONE exp for both heads (halves ScalarE call
                            # overhead); for diagonal tiles the [0:c0) region
                            # is untouched-PSUM garbage in pt, but it is
                            # either memset below (jl==0) or never read by
                            # the narrowed AV matmul (jl>0)
                            nc.scalar.activation(
                                out=pt[:, :, c0:], in_=ss[:, :, c0:],
                                func=mybir.ActivationFunctionType.Exp)
                            if jl == 0 and c0 > 0:
                                # AV stays full-width for jl==0
                                nc.vector.memset(pt[:, :, 0:c0], 0.0)
                            if r >= 0:
                                m0 = TKT * r
                                for h in range(HPC):
                                    nc.vector.tensor_mul(
                                        pt[:, h, m0:m0 + TKT],
                                        pt[:, h, m0:m0 + TKT],
                                        masks_sb[:, r, m0:m0 + TKT])
                            pts[kt] = pt
                        for i, kt in enumerate(kts):
                            vt = v_sb[(T // 128) * b + kt]
                            r = kt - dg * jl
                            c0 = TKT * r if (jl > 0 and r >= 1) else 0
                            del r
                            for h in range(HPC):
                                nc.tensor.matmul(
                                    y_ps[h][:, c0:],
                                    lhsT=vt[:, (D + 1) * h:(D + 1) * (h + 1)],
                                    rhs=pts[kt][:, h, c0:],
                                    start=(i == 0), stop=(i == len(kts) - 1))
                        for h in range(HPC):
                            den = npool.tile([1, TQ], F32, tag="den")
                            nc.vector.tensor_copy(out=den,
                                                  in_=y_ps[h][D:D + 1, :])
                            recip = npool.tile([1, TQ], F32, tag="recip")
                            nc.vector.reciprocal_approx_fast(recip, den)
                            recip_b = npool.tile([D, TQ], F32, tag="recipb")
                            nc.gpsimd.partition_broadcast(recip_b, recip)
                            nc.vector.tensor_mul(ytc[h][:, j, :],
                                                 y_ps[h][0:D, :], recip_b)
                            if last_sc:
                                nc.sync.dma_start(
                                    out=a2a_in_f[j][:, D * h:D * (h + 1), :]
                                        .rearrange("s p q -> p s q"),
                                    in_=ytc[h][:, j, :]
                                        .rearrange("p (s q) -> p s q", s=SPC))
                            else:
                                nc.sync.dma_start(
                                    out=a2a_in[sc][:, D * h:D * (h + 1), j, :]
                                        .rearrange("s p q -> p s q"),
                                    in_=ytc[h][:, j, :]
                                        .rearrange("p (s q) -> p s q", s=SPC))
                        if j == 1 and not last_sc:
                            nc.gpsimd.collective_compute(
                                "AllToAll", mybir.AluOpType.bypass,
                                replica_groups=[list(range(NCORES))],
                                ins=[a2a_in[sc].opt()],
                                outs=[a2a_out[sc].opt()])
                            pending.append(sc)
                        elif j == 1:
                            yyf = opool.tile([128, KT, 128], BF16,
                                             tag="ylhs", name="ylhsf")
                            with tc.tile_wait_until(S4F_FLOOR[0]):
                                stage4_final_half(yyf, pos, 0)
                            nc.gpsimd.collective_compute(
                                "AllToAll", mybir.AluOpType.bypass,
                                replica_groups=[list(range(NCORES))],
                                ins=[a2a_in_f[1].opt()],
                                outs=[a2a_out_f[1].opt()])
                            if pending:
                                pop_stage4()
                            with tc.tile_wait_until(S4F_FLOOR[1]):
                                stage4_final_half(yyf, pos, 1)

            for sc in pending:
                with tc.tile_wait_until(S4_FLOOR[sc]):
                    stage4(sc)

    nc.compile()
    return nc


_lock = threading.Lock()
_cached_nc = None
last_results = None  # BassKernelResults of the most recent kernel() call


def _get_program():
    global _cached_nc
    with _lock:
        if _cached_nc is None:
            _cached_nc = _build_program()
    return _cached_nc


def _host_inputs(x, W_qkv, b_qkv, W_proj, b_proj):
    bf = lambda a: np.ascontiguousarray(a).astype(BF16_NP)
    x = np.asarray(x, dtype=np.float32)
    W_qkv = np.asarray(W_qkv, dtype=np.float32)
    b_qkv = np.asarray(b_qkv, dtype=np.float32)
    W_proj = np.asarray(W_proj, dtype=np.float32)
    b_proj = np.asarray(b_proj, dtype=np.float32)

    xt = bf(x.reshape(BT, C).T)                     # [C, BT]
    wproj = bf(W_proj)                              # [C, C]
    bproj = bf(b_proj.reshape(1, C))
    r = np.arange(TQ // TKT)[:, None, None]
    k = np.arange(TKT)[None, :, None]
    q = np.arange(TQ)[None, None, :]
    masks = ((k + TKT * r) <= q).astype(BF16_NP)    # [4, 128, 512]

    in_maps = []
    for i in range(NCORES):
        sel = slice(HC * i, HC * (i + 1))
        wq = W_qkv[:, sel]
        wk = W_qkv[:, C + HC * i:C + HC * (i + 1)] * SM_SCALE
        wv = W_qkv[:, 2 * C + HC * i:2 * C + HC * (i + 1)]
        in_maps.append({
            "xt": xt,
            "wqkv": bf(np.concatenate([wq, wk, wv], axis=1)),
            "wproj": wproj,
            "bq": np.ascontiguousarray(
                b_qkv[sel].reshape(HC, 1)).astype(np.float32),
            "bk": np.ascontiguousarray(
                (b_qkv[C + HC * i:C + HC * (i + 1)] * SM_SCALE)
                .reshape(HC, 1)).astype(np.float32),
            "bv": b_qkv[2 * C + HC * i:2 * C + HC * (i + 1)]
                .reshape(1, HC).astype(BF16_NP),
            "bproj": bproj,
            "masks": masks,
        })
    return in_maps


def kernel(x, W_qkv, b_qkv, W_proj, b_proj):
    global last_results
    nc = _get_program()
    in_maps = _host_inputs(x, W_qkv, b_qkv, W_proj, b_proj)
    trace = bool(int(os.environ.get("KERNEL_TRACE", "0")))
    res = bass_utils.run_bass_kernel_spmd(
        nc, in_maps, core_ids=list(range(NCORES)), trace=trace)
    last_results = res
    # core s's output rows are strip s (64 rows) of every 512-row chunk
    arr = np.stack([res.results[s]["out"].reshape(BT // TQ, D, C)
                    for s in range(NCORES)], axis=1)   # [chunk, core, 64, C]
    return np.ascontiguousarray(arr.reshape(B, T, C)).astype(np.float32)
